# revision 74
# baseline (speedup 1.0000x reference)
"""GQA attention (BagleyAttention) on 8 Trainium2 NeuronCores.

Tensor-parallel over kv-head groups: core c owns kv head c and query heads
[4c, 4c+4). Each core computes its heads' attention and a partial output
projection [S, D]; the host sums the 8 partials.

v4: PSUM-pressure + ACT-overhead rework over v3 (403us -> ~389us):
  - projection rounds 1-3 run in two 256-column group-major passes with 6
    half-bank accumulators packed into 3 PSUM banks (a start=True matmul
    invalidates the FULL psum rows of its bank, so groups sharing a bank
    must accumulate contiguously -- never dcg-major); round 0 (x/weights
    still streaming in) runs full-width dc-major using the then-idle
    score/pv banks as 6 separate single-bank accumulators;
  - scores accumulate into a 2-bank [128,1024] PSUM tile; full k-chunks
    are exp'd in PAIRS with a single ACTIVATE (halves the 352-cycle fixed
    cost per instruction); in the projection-free final round the pab
    projection tile doubles as a second score pair-tile so the sc-write
    WAR never serializes against the exp chain;
  - the softmax denominator accumulates in ONE wide fp16 tile, DVE for
    pairs, idle GpSimd for the trailing diagonal chunks, so the Z-matmul
    never queues behind outproj copies; ob copies are split ACT/DVE per
    round with ACT kept clear wherever the exp chain is dense;
  - RoPE rotate-half runs on DVE stream_shuffle in a host-permuted head-dim
    basis (pairs (j, j+64) interleaved on adjacent partitions; swap mask
    i^1 is quadrant-local; the sign lives in the partner-indexed sin table
    and cos[j] == cos[j+64] for this rope) -> no PE permutation matmuls,
    no ACT PSUM->SBUF copies, and the projection accumulators are freed by
    two DVE reads;
  - outproj PSUM rotation folds idle projection banks in during the final
    rounds; x staging for round r+1 is queued right after round r's passB,
    ahead of the output-store DMAs.
"""

import math
import sys

sys.path.insert(0, "/opt/trn_rl_repo")

import numpy as np

# Problem sizes (hardcoded per contract; kernel.py reads no sibling files).
B, S, D = 1, 2048, 4096
H, KV, Dh = 32, 8, 128
G = H // KV            # query heads per kv head (= per core)
EH = G * Dh            # per-core q projection width (512)
N_CORES = 8

SB = 512               # s-block width (projection s-block = attention q-block)
HB = 256               # projection half-block (column pass width)
N_SB = S // SB         # 4 rounds
N_DC = D // 128        # 32 d-chunks
N_DCG = 8              # d-chunk groups of 4 (x staging granularity)
N_NB = D // SB         # 8 output d-blocks

EXP_BIAS = 9.5         # exp(s - EXP_BIAS); cancels in softmax normalization

SWAP_MASK = [i ^ 1 for i in range(32)]   # even/odd partition swap (per quad)

_cache = {}


def _build():
    import concourse.bass as bass
    import concourse.mybir as mybir
    import concourse.tile as tile
    from concourse import bacc
    from concourse.masks import make_identity

    dt = mybir.dt
    f32, f16 = dt.float32, dt.float16
    AF = mybir.ActivationFunctionType

    nc = bacc.Bacc("TRN2", target_bir_lowering=False, debug=False)

    # host-pretiled inputs (see _prep_inputs for layouts)
    xg = nc.dram_tensor("xg", [N_SB, N_DCG, 128, 4, SB], f16,
                        kind="ExternalInput").ap()
    wqt = nc.dram_tensor("wqt", [128, N_DC, EH], f16,
                         kind="ExternalInput").ap()
    wkt = nc.dram_tensor("wkt", [128, N_DC, Dh], f16,
                         kind="ExternalInput").ap()
    wvt = nc.dram_tensor("wvt", [128, N_DC, Dh], f16,
                         kind="ExternalInput").ap()
    wot = nc.dram_tensor("wot", [128, G, D], f16, kind="ExternalInput").ap()
    cosT = nc.dram_tensor("cosT", [Dh, S], f16, kind="ExternalInput").ap()
    sinT = nc.dram_tensor("sinT", [Dh, S], f16, kind="ExternalInput").ap()
    triT = nc.dram_tensor("triT", [128, 128], f16, kind="ExternalInput").ap()
    out = nc.dram_tensor("out", [S, D], f16, kind="ExternalOutput").ap()

    inv_sqrt_dh = 1.0 / math.sqrt(Dh)

    with tile.TileContext(nc) as tc, \
         tc.tile_pool(name="persist", bufs=1) as persist, \
         tc.tile_pool(name="projp", bufs=1, space="PSUM") as projp, \
         tc.tile_pool(name="scp", bufs=1, space="PSUM") as scp, \
         tc.tile_pool(name="pvp", bufs=1, space="PSUM") as pvp, \
         tc.tile_pool(name="trans", bufs=2, space="PSUM") as trans, \
         tc.tile_pool(name="xstage", bufs=10) as xstage, \
         tc.tile_pool(name="ropep", bufs=4) as ropep, \
         tc.tile_pool(name="expp", bufs=2) as expp, \
         tc.tile_pool(name="eaccp", bufs=2) as eaccp, \
         tc.tile_pool(name="miscp", bufs=2) as miscp, \
         tc.tile_pool(name="obuf", bufs=2) as obuf:

        # ---- long-lived SBUF tensors --------------------------------------
        qr = [[persist.tile([128, SB], f16, tag=f"qr{h}_{sb}",
                            name=f"qr{h}_{sb}") for sb in range(N_SB)]
              for h in range(G)]
        kr = [persist.tile([128, SB], f16, tag=f"kr{sb}", name=f"kr{sb}")
              for sb in range(N_SB)]
        vnat = [persist.tile([128, N_SB * Dh], f16, tag=f"vn{sb}",
                             name=f"vn{sb}") for sb in range(N_SB)]
        attn = [[persist.tile([128, SB], f16, tag=f"attn{h}_{t}",
                              name=f"attn{h}_{t}") for t in range(N_SB)]
                for h in range(G)]

        wq_h = persist.tile([128, N_DC, EH], f16, tag="wq_h", name="wq_h")
        wk_h = persist.tile([128, N_DC, Dh], f16, tag="wk_h", name="wk_h")
        wv_h = persist.tile([128, N_DC, Dh], f16, tag="wv_h", name="wv_h")
        wo_r = persist.tile([128, G, D], f16, tag="wo_r", name="wo_r")
        cos_sb = persist.tile([128, S], f16, tag="cos", name="cos_sb")
        sin_sb = persist.tile([128, S], f16, tag="sin", name="sin_sb")
        tri = persist.tile([128, 128], f16, tag="tri", name="tri")
        ones_h = persist.tile([128, 128], f16, tag="ones", name="ones_h")
        ident = persist.tile([128, 128], f16, tag="ident", name="ident")
        ebias = persist.tile([128, 1], f32, tag="ebias", name="ebias")

        nc.vector.memset(ones_h, 1.0)
        nc.vector.memset(ebias, -EXP_BIAS)
        make_identity(nc, ident)

        # PE warm-up: a few no-dep matmuls run during the initial DMA wait,
        # priming the HAM activity window so the first projection matmuls
        # start closer to the full 2.4 GHz clock. Measured sweep: more than
        # a handful costs real stream time (~285ns each) and is net-negative.
        wsrc = persist.tile([128, SB], f16, tag="wsrc", name="wsrc")
        nc.vector.memset(wsrc, 0.000244140625)
        warm = trans.tile([128, SB], f32, tag="tr", name="warm")
        for _ in range(6):
            nc.tensor.matmul(warm[:], ones_h[:], wsrc[:],
                             start=True, stop=True)

        vt_sb = persist.tile([128, SB], f16, tag="vt", name="vt_sb")

        # Projection accumulators: 6 groups x [128, HB] packed as 3 banks.
        # pab is a 2-bank tile (groups 0-3) so that in the projection-free
        # final rounds it can double as a SECOND score pair-tile.
        pab = projp.tile([128, 2 * SB], f32, tag="pab", name="pab")
        pc2 = projp.tile([128, SB], f32, tag="pc2", name="pc2")

        def acc_slot(g):
            if g < 4:
                return pab[:, g * HB:(g + 1) * HB]
            return pc2[:, (g - 4) * HB:(g - 3) * HB]

        # Scores: one 2-bank tile; chunk c uses half c%2.
        sc2 = scp.tile([128, 2 * SB], f32, tag="sc2", name="sc2")
        # pv accumulator (one bank, WAR-rotated across (t,h) blocks)
        pv = pvp.tile([128, SB], f32, tag="pv", name="pv")

        def sct(t, pi):
            # score pair-tile for chunk-pair index pi: alternate with the
            # (idle) pab banks in the final attention round so the sc WAR
            # never serializes against the exp chain
            if t == N_SB - 1 and pi % 2 == 1:
                return pab
            return sc2

        # -------------------------------------------------------------------
        # DMA emission (all host-pretiled, big per-partition rows)
        # -------------------------------------------------------------------
        xq_tiles = {}   # (sb, dcg) -> tile [128, 4, SB]

        def emit_x_dma(sb):
            for dcg in range(N_DCG):
                xq = xstage.tile([128, 4, SB], f16, tag="xq",
                                 name=f"xq{sb}_{dcg}")
                nc.sync.dma_start(out=xq, in_=xg[sb, dcg])
                xq_tiles[(sb, dcg)] = xq

        def emit_weight_dma():
            def wq_g(g):
                cs = slice(g * 8, (g + 1) * 8)
                nc.sync.dma_start(out=wq_h[:, cs, :], in_=wqt[:, cs, :])

            def wkv_g(g):
                cs = slice(g * 16, (g + 1) * 16)
                nc.sync.dma_start(out=wk_h[:, cs, :], in_=wkt[:, cs, :])
                nc.sync.dma_start(out=wv_h[:, cs, :], in_=wvt[:, cs, :])

            def xq_g(dcg):
                xq = xstage.tile([128, 4, SB], f16, tag="xq",
                                 name=f"xq0_{dcg}")
                nc.sync.dma_start(out=xq, in_=xg[0, dcg])
                xq_tiles[(0, dcg)] = xq

            # small first descriptors so the first matmul starts ASAP
            nc.sync.dma_start(out=wq_h[:, 0:1, :], in_=wqt[:, 0:1, :])
            nc.sync.dma_start(out=wk_h[:, 0:1, :], in_=wkt[:, 0:1, :])
            nc.sync.dma_start(out=wv_h[:, 0:1, :], in_=wvt[:, 0:1, :])
            xq_g(0)
            nc.sync.dma_start(out=wq_h[:, 1:2, :], in_=wqt[:, 1:2, :])
            nc.sync.dma_start(out=wk_h[:, 1:4, :], in_=wkt[:, 1:4, :])
            nc.sync.dma_start(out=wv_h[:, 1:4, :], in_=wvt[:, 1:4, :])
            nc.sync.dma_start(out=wq_h[:, 2:8, :], in_=wqt[:, 2:8, :])
            nc.sync.dma_start(out=wk_h[:, 4:8, :], in_=wkt[:, 4:8, :])
            nc.sync.dma_start(out=wv_h[:, 4:8, :], in_=wvt[:, 4:8, :])
            xq_g(1)
            nc.sync.dma_start(out=wk_h[:, 8:16, :], in_=wkt[:, 8:16, :])
            nc.sync.dma_start(out=wv_h[:, 8:16, :], in_=wvt[:, 8:16, :])
            wq_g(1); xq_g(2); xq_g(3)
            wq_g(2); wkv_g(1); xq_g(4); xq_g(5)
            wq_g(3); xq_g(6); xq_g(7)
            # trig/masks land before their first readers and must be EMITTED
            # before those readers too, else Tile orders the load after them.
            nc.sync.dma_start(out=cos_sb, in_=cosT)
            nc.sync.dma_start(out=sin_sb, in_=sinT)
            nc.sync.dma_start(out=tri, in_=triT)

        # -------------------------------------------------------------------
        # Projection pass: 6 groups (4 q heads, k, v) x 32 d-chunks over one
        # 256-column half. Group-major so group g's accumulator is complete
        # (and drainable) 5/6 of a pass before the pass ends.
        # -------------------------------------------------------------------
        def wsel(g, dc):
            if g < G:
                return wq_h[:, dc, g * 128:(g + 1) * 128]
            if g == 4:
                return wk_h[:, dc, :]
            return wv_h[:, dc, :]

        # NOTE on ordering: a start=True matmul invalidates the has_written
        # state for the FULL psum rows of its bank, so two accumulation
        # groups sharing a bank must never interleave their matmuls.
        # Group-major keeps each group contiguous (and lets its rope drain
        # run during the pass); it is the only legal order for the packed
        # half-bank accumulators.
        def emit_proj_pass(r, half):
            o = half * HB
            for g in range(6):
                dst = acc_slot(g)
                for dcg in range(N_DCG):
                    xf = xq_tiles[(r, dcg)]
                    for j in range(4):
                        dc = dcg * 4 + j
                        nc.tensor.matmul(dst, wsel(g, dc), xf[:, j, o:o + HB],
                                         start=(dc == 0),
                                         stop=(dc == N_DC - 1))

        # Round 0: x/weights stream in over ~20us, so group-major would
        # starve on DMA. With no attention live yet, the sc2/pv banks are
        # free: run the baseline-style full-width dc-major pass with 6
        # accumulators in 6 SEPARATE banks (cross-group interleave across
        # different banks is safe).
        def emit_proj_full_r0():
            accs = [pab[:, 0:SB], pab[:, SB:2 * SB], pc2[:],
                    sc2[:, 0:SB], sc2[:, SB:2 * SB], pv[:]]
            # dc-major through dc 27 (matches DMA arrival), then finish the
            # groups one at a time so their rope drains start staggered
            # instead of all at the pass boundary
            for dcg in range(N_DCG - 1):
                xf = xq_tiles[(0, dcg)]
                for j in range(4):
                    dc = dcg * 4 + j
                    for g in range(6):
                        nc.tensor.matmul(accs[g], wsel(g, dc), xf[:, j, :],
                                         start=(dc == 0), stop=False)
            xf = xq_tiles[(0, N_DCG - 1)]
            for g in range(6):
                for j in range(4):
                    dc = (N_DCG - 1) * 4 + j
                    nc.tensor.matmul(accs[g], wsel(g, dc), xf[:, j, :],
                                     start=False, stop=(dc == N_DC - 1))
            return accs

        # RoPE for one (slot, half): all DVE + one GpSimd add; the PSUM
        # accumulator is freed after the two DVE reads. sin_sb is the
        # PARTNER-indexed signed sin table so the multiply happens before
        # the (same-dtype) stream_shuffle:
        #   rot[p] = acc[p^1]*sinM[p] = shuffle(acc*sinP)[p],
        #   sinP[p] = sinM[p^1].
        def make_rope_units(r, half, accs=None, W=HB):
            o = half * HB
            ss = slice(r * SB + o, r * SB + o + W)

            def get_acc(i):
                return accs[i] if accs is not None else acc_slot(i)

            def rope(i):
                acc = get_acc(i)
                c_ = ropep.tile([128, W], f16, tag="rc", name=f"rc{r}_{i}")
                s_ = ropep.tile([128, W], f16, tag="rs", name=f"rs{r}_{i}")
                w_ = ropep.tile([128, W], f16, tag="rw", name=f"rw{r}_{i}")
                nc.vector.tensor_mul(c_, acc, cos_sb[:, ss])
                nc.vector.tensor_mul(s_, acc, sin_sb[:, ss])
                nc.vector.stream_shuffle(w_, s_, SWAP_MASK)
                dst = qr[i][r] if i < G else kr[r]
                nc.gpsimd.tensor_add(dst[:, o:o + W], c_, w_)

            def vdrain():
                nc.scalar.copy(out=vt_sb[:, o:o + W], in_=get_acc(5))

            return [lambda i=i: rope(i) for i in range(5)] + [vdrain]

        # V natural layout: one XBAR DMA transpose per 128-col block --
        # entirely off the PE/DVE (the consumer is a full round away).
        def emit_transpose(r, blk):
            nc.sync.dma_start_transpose(
                out=vnat[r][:, blk * Dh:(blk + 1) * Dh],
                in_=vt_sb[:, blk * 128:(blk + 1) * 128])

        # -------------------------------------------------------------------
        # Attention for q-block t, head h. Chunks are emitted as units:
        #   ("pair", t, h, p): full chunks 2p, 2p+1 -> 2 sc MMs, 1 paired
        #       exp, 1 wide DVE e-add, 2 pv MMs
        #   ("solo", t, h, c): diagonal chunk -> sc MM, exp, mask, add, pv
        # -------------------------------------------------------------------
        att_state = {}

        def attn_begin(t, h):
            st = dict(n=4 * (t + 1))
            st["ed"] = eaccp.tile([128, 2 * SB], f16, tag="ed", name="ed")
            att_state[(t, h)] = st

        def attn_pair(t, h, p):
            st = att_state[(t, h)]
            sc = sct(t, p)
            c0 = 2 * p
            for ci in range(2):
                c = c0 + ci
                nc.tensor.matmul(
                    sc[:, ci * SB:(ci + 1) * SB],
                    kr[c // 4][:, (c % 4) * 128:(c % 4) * 128 + 128],
                    qr[h][t][:], start=True, stop=True)
            e = expp.tile([128, 2 * SB], f16, tag="e", name="e")
            nc.scalar.activation(e[:], sc[:], AF.Exp,
                                 scale=inv_sqrt_dh, bias=ebias[:])
            if p == 0:
                nc.vector.tensor_copy(st["ed"][:], e[:])
            else:
                nc.vector.tensor_add(st["ed"][:], st["ed"][:], e[:])
            for ci in range(2):
                c = c0 + ci
                vw = vnat[c // 4][:, (c % 4) * Dh:(c % 4 + 1) * Dh]
                nc.tensor.matmul(pv[:], vw, e[:, ci * SB:(ci + 1) * SB],
                                 start=(c == 0), stop=(c == st["n"] - 1))

        def attn_solo(t, h, c):
            st = att_state[(t, h)]
            sc = sct(t, c // 2)
            hf = (c % 2) * SB
            qlo = 128 * (c - 4 * t)
            W = SB - qlo
            nc.tensor.matmul(sc[:, hf:hf + W],
                             kr[c // 4][:, (c % 4) * 128:(c % 4) * 128 + 128],
                             qr[h][t][:, qlo:SB], start=True, stop=True)
            e = expp.tile([128, 2 * SB], f16, tag="e", name="e")
            nc.scalar.activation(e[:, hf:hf + W], sc[:, hf:hf + W], AF.Exp,
                                 scale=inv_sqrt_dh, bias=ebias[:])
            # diagonal: mask first 128 q-cols (on the pv critical path)
            nc.vector.tensor_mul(e[:, hf:hf + 128], e[:, hf:hf + 128], tri[:])
            ed = st["ed"]
            if c <= 1:
                # t == 0: ed halves first written by chunks 0 (full) / 1
                if c == 1:
                    nc.vector.memset(ed[:, SB:SB + 128], 0.0)
                nc.vector.tensor_copy(ed[:, hf + qlo:hf + SB], e[:, hf:hf + W])
            else:
                # in the proj-free final round GpSimd is idle: trailing solos
                # accumulate there so the Z-matmul doesn't queue behind the
                # DVE copies; in earlier rounds GpSimd runs the rope combines
                # and would be slower than DVE
                on_gp = (t == N_SB - 1) and c >= st["n"] - 2
                eng = nc.gpsimd if on_gp else nc.vector
                eng.tensor_add(ed[:, hf + qlo:hf + SB],
                               ed[:, hf + qlo:hf + SB], e[:, hf:hf + W])
            vw = vnat[c // 4][:, (c % 4) * Dh:(c % 4 + 1) * Dh]
            nc.tensor.matmul(pv[:, qlo:SB], vw, e[:, hf:hf + W],
                             start=(c == 0), stop=(c == st["n"] - 1))

        def attn_eplg(t, h):
            st = att_state.pop((t, h))
            zb = trans.tile([128, SB], f32, tag="tr", name=f"z{t}_{h}")
            nc.tensor.matmul(zb[:], ones_h[:], st["ed"][:, 0:SB],
                             start=True, stop=False)
            nc.tensor.matmul(zb[:], ones_h[:], st["ed"][:, SB:2 * SB],
                             start=False, stop=True)
            rz = miscp.tile([128, SB], f32, tag="rz", name="rz")
            nc.vector.reciprocal_approx_fast(out=rz, in_=zb[:])
            nc.vector.tensor_mul(attn[h][t][:], pv[:], rz[:])

        # -------------------------------------------------------------------
        # Output projection: 8 groups of 4 d-blocks per q-block; one DMA
        # per group ([128, 2048] = 4KB rows).
        # -------------------------------------------------------------------
        ob_state = {}

        def outproj_tile(t, i, ob_eng, deep_psum=0):
            sl = (i // N_NB) * 128          # s-offset within block
            nb = i % N_NB
            st_row = 4 * t + i // N_NB
            if t == 3 and i >= 24:
                # strict alternation at the very end: neither engine's queue
                # may delay the final copies ahead of the closing DMAs
                ob_eng = "act" if i % 2 else "dve"
            # in the proj-free rounds idle PSUM banks join the rotation so
            # the PE never waits on the ob copies. Round 4: pc2 (pab is the
            # alternate score tile there). Round 5: pc2 + both sc2 halves.
            if deep_psum == 2 and i % 5 == 2:
                op = pc2[:]
            elif deep_psum == 2 and i % 5 == 3:
                op = sc2[:, 0:SB]
            elif deep_psum == 2 and i % 5 == 4:
                op = sc2[:, SB:2 * SB]
            elif deep_psum == 1 and i % 3 == 2:
                op = pc2[:]
            else:
                op = trans.tile([128, SB], f32, tag="tr", name=f"op{t}_{i}")
            for hh in range(G):
                nc.tensor.matmul(op[:], attn[hh][t][:, sl:sl + 128],
                                 wo_r[:, hh, nb * SB:(nb + 1) * SB],
                                 start=(hh == 0), stop=(hh == G - 1))
            # group width: 4 d-blocks per DMA; narrower at the very end so
            # the final transfers start earlier (shorter kernel tail)
            w = 1 if (t == 3 and i >= 28) else 2 if (t == 3 and i >= 24) else 4
            if nb % w == 0:
                ob_state[st_row] = obuf.tile([128, w * SB], f16, tag="ob",
                                             name=f"ob{t}_{i}")
            ob = ob_state[st_row]
            qtr = nb % w
            if ob_eng == "act":
                nc.scalar.copy(out=ob[:, qtr * SB:(qtr + 1) * SB], in_=op[:])
            else:
                nc.vector.tensor_copy(ob[:, qtr * SB:(qtr + 1) * SB], op[:])
            if nb % w == w - 1:
                rs = slice(st_row * 128, (st_row + 1) * 128)
                cs = slice((nb - w + 1) * SB, (nb + 1) * SB)
                nc.sync.dma_start(out=out[rs, cs], in_=ob[:, 0:w * SB])
                del ob_state[st_row]

        # -------------------------------------------------------------------
        # Round schedule:
        #   r0: proj0|rope0       r1: proj1|rope1|attn0
        #   r2: proj2|rope2|attn1|outproj0   r3: proj3|rope3|attn2|outproj1
        #   r4: attn3|outproj2    r5: outproj3
        # -------------------------------------------------------------------
        # ob-copy share on ACT: ACT must keep slack for the exps (the sc2
        # pair rotation makes the PE's sc stream wait on exp completion);
        # round 4 is exp-densest so ACT gets no copies at all there
        ACT_OB_SHARE = {2: 0.4, 3: 0.45, 4: 0.0, 5: 0.55}

        def emit_round(r):
            ta = r - 1            # attention q-block this round
            to = r - 2            # out-projection q-block this round

            # attention PE-work units for this round, in order
            units = []
            if 0 <= ta < N_SB:
                n = 4 * (ta + 1)
                for h in range(G):
                    units.append(("begin", ta, h))
                    for p in range(2 * ta):
                        units.append(("pair", ta, h, p))
                    for c in range(4 * ta, n):
                        units.append(("solo", ta, h, c))
                    units.append(("eplg", ta, h))
            nop = 32 if 0 <= to < N_SB else 0
            nch = sum(1 for u in units if u[0] in ("pair", "solo"))

            ropes0 = []
            ropesA = []
            if r == 0:
                accs0 = emit_proj_full_r0()
                emit_x_dma(1)
                ropes0 = make_rope_units(0, 0, accs=accs0, W=SB)
            elif r < N_SB:
                emit_proj_pass(r, 0)
                ropesA = make_rope_units(r, 0)

            # Post-passA stream. With group-major passes, rope unit g's PSUM
            # deps complete (g+1)/6 of the way through the pass, so rope
            # units emitted before passB execute DURING the pass itself and
            # passB never waits on accumulator drains.
            seq = []
            ui = 0
            if r == 0:
                # v transposes deferred into round 1: they wait on the ACT
                # vdrain and would stall the PE right before round 1's passA
                seq += [("rope0", k) for k in range(6)]
            elif r < N_SB:
                # round 0's deferred transposes must precede ropeA: round
                # 1's vdrain overwrites the shared vt_sb staging buffer
                if r == 1:
                    seq += [("transp", 0, b) for b in range(4)]
                seq += [("ropeA", k) for k in range(6)]
                seq.append(("passB", r))
                seq.append(("transp", r, 0))
                seq.append(("ropesB_make", r))
                # interleave ropeB with the first attn units so the v-half1
                # transposes (which need ropeB's vdrain) come a bit later
                for k in range(6):
                    seq.append(("ropeB", k))
                    if ui < len(units):
                        seq.append(units[ui]); ui += 1
                seq.append(("transp", r, 1))
                seq.append(("transp", r, 2))
                seq.append(("transp", r, 3))
            op_i = 0
            ob_flip = 0.0
            chunk_seen = sum(1 for u in seq if u[0] in ("pair", "solo"))
            while ui < len(units):
                u = units[ui]; ui += 1
                if u[0] == "eplg":
                    # cover the Z-matmul's E-accumulator wait; force these
                    # copies onto ACT so the DVE queue stays short ahead of
                    # the Z-matmul's ed dependency
                    for _ in range(2):
                        if op_i < nop:
                            seq.append(("opact", to, op_i))
                            op_i += 1
                seq.append(u)
                if u[0] in ("pair", "solo"):
                    chunk_seen += 1
                    while nch and op_i < nop and (op_i + 1) / nop <= \
                            chunk_seen / nch:
                        seq.append(("op", to, op_i))
                        op_i += 1
            while op_i < nop:
                seq.append(("op", to, op_i))
                op_i += 1

            act_share = ACT_OB_SHARE.get(r, 0.0)
            ropesB = []
            for u in seq:
                kind = u[0]
                if kind == "begin":
                    attn_begin(u[1], u[2])
                elif kind == "pair":
                    attn_pair(u[1], u[2], u[3])
                elif kind == "solo":
                    attn_solo(u[1], u[2], u[3])
                elif kind == "eplg":
                    attn_eplg(u[1], u[2])
                elif kind == "passB":
                    emit_proj_pass(u[1], 1)
                    # queue next round's x staging ahead of this round's
                    # output-store DMAs so the next passA never starves
                    if u[1] + 1 < N_SB:
                        emit_x_dma(u[1] + 1)
                    if u[1] == 1:
                        for g in range(G):
                            nc.sync.dma_start(out=wo_r[:, g, :],
                                              in_=wot[:, g, :])
                elif kind == "rope0":
                    ropes0[u[1]]()
                elif kind == "ropeA":
                    ropesA[u[1]]()
                elif kind == "ropesB_make":
                    ropesB[:] = make_rope_units(u[1], 1)
                elif kind == "ropeB":
                    ropesB[u[1]]()
                elif kind == "transp":
                    emit_transpose(u[1], u[2])
                elif kind == "opact":
                    outproj_tile(u[1], u[2], "dve",
                                 deep_psum=max(0, r - N_SB + 1))
                elif kind == "op":
                    ob_flip += act_share
                    if ob_flip >= 1.0:
                        ob_flip -= 1.0
                        eng = "act"
                    else:
                        eng = "dve"
                    outproj_tile(u[1], u[2], eng,
                                 deep_psum=max(0, r - N_SB + 1))

        emit_weight_dma()
        for r in range(N_SB + 2):
            emit_round(r)
            # pop x staging after the round that consumed it
            if r < N_SB:
                for dcg in range(N_DCG):
                    xq_tiles.pop((r, dcg), None)

    nc.compile()
    return nc


def _prep_inputs(hidden_states, Wq, Wk, Wv, Wo, cos, sin):
    x = np.asarray(hidden_states, dtype=np.float32).reshape(S, D)
    Wq = np.asarray(Wq, dtype=np.float32)
    Wk = np.asarray(Wk, dtype=np.float32)
    Wv = np.asarray(Wv, dtype=np.float32)
    Wo = np.asarray(Wo, dtype=np.float32)
    cos = np.asarray(cos, dtype=np.float32)
    sin = np.asarray(sin, dtype=np.float32)

    # Head-dim basis permutation: partition 2j holds element j, partition
    # 2j+1 holds element j+64 -> the rope pair sits on adjacent partitions
    # and rotate-half becomes a quadrant-local even/odd stream_shuffle.
    half = Dh // 2
    perm = np.empty(Dh, dtype=np.int64)
    perm[0::2] = np.arange(half)
    perm[1::2] = np.arange(half) + half

    # x pretiled: xg[sb, dcg, p, j, s] = x.T[dcg*512 + j*128 + p, sb*512 + s]
    xT = np.ascontiguousarray(x.T).astype(np.float16)
    xg = np.ascontiguousarray(
        xT.reshape(N_DCG, 4, 128, N_SB, SB).transpose(3, 0, 2, 1, 4))
    # cos in permuted basis; sin with the rotate-half sign folded in:
    # row 2j   (elem j):    needs -sin[j]   * (partner value)
    # row 2j+1 (elem j+64): needs +sin[j+64]* (partner value)
    # The kernel multiplies BEFORE shuffling, so ship the partner-indexed
    # table sinP[p] = sinM[p^1].
    cosM = cos.T[perm, :]
    sinM = sin.T[perm, :].copy()
    sinM[0::2, :] *= -1.0
    swap = np.arange(Dh) ^ 1
    sinP = sinM[swap, :]
    cosT_h = np.ascontiguousarray(cosM).astype(np.float16)
    sinT_h = np.ascontiguousarray(sinP).astype(np.float16)
    # lower-triangle (inclusive) 0/1 mask for the 128x128 diagonal block
    kp = np.arange(128)[:, None]
    qc = np.arange(128)[None, :]
    triT = (kp <= qc).astype(np.float16)

    in_maps = []
    for c in range(N_CORES):
        wq_s = Wq[c * EH:(c + 1) * EH, :]          # [EH, D]
        wk_s = Wk[c * Dh:(c + 1) * Dh, :]
        wv_s = Wv[c * Dh:(c + 1) * Dh, :]
        wo_s = Wo[:, c * EH:(c + 1) * EH]          # [D, EH]
        # permute q/k head-dim rows into the interleaved rope basis
        wq_p = wq_s.reshape(G, Dh, D)[:, perm, :].reshape(EH, D)
        wk_p = wk_s[perm, :]
        # wqt[p, dc, e] = wq_p.T[dc*128+p, e]
        wqt = np.ascontiguousarray(
            np.ascontiguousarray(wq_p.T).astype(np.float16)
            .reshape(N_DC, 128, EH).transpose(1, 0, 2))
        wkt = np.ascontiguousarray(
            np.ascontiguousarray(wk_p.T).astype(np.float16)
            .reshape(N_DC, 128, Dh).transpose(1, 0, 2))
        wvt = np.ascontiguousarray(
            np.ascontiguousarray(wv_s.T).astype(np.float16)
            .reshape(N_DC, 128, Dh).transpose(1, 0, 2))
        # wot[p, h, d] = wo_s.T[h*128+p, d]
        wot = np.ascontiguousarray(
            np.ascontiguousarray(wo_s.T).astype(np.float16)
            .reshape(G, 128, D).transpose(1, 0, 2))
        in_maps.append({
            "xg": xg, "wqt": wqt, "wkt": wkt, "wvt": wvt, "wot": wot,
            "cosT": cosT_h, "sinT": sinT_h, "triT": triT,
        })
    return in_maps


def run(trace=False, **inputs):
    """Run on hardware; returns (full_output, exec_time_ns or None)."""
    from concourse.bass_utils import run_bass_kernel_spmd

    if trace:
        _install_ntff_hook()
    if "nc" not in _cache:
        _cache["nc"] = _build()
    nc = _cache["nc"]
    in_maps = _prep_inputs(**inputs)
    res = run_bass_kernel_spmd(nc, in_maps, core_ids=list(range(N_CORES)),
                               trace=trace)
    acc = res.results[0]["out"].astype(np.float32)
    for c in range(1, N_CORES):
        acc += res.results[c]["out"]
    return acc.reshape(B, S, D), res.exec_time_ns


def _install_ntff_hook():
    """Register the axon NTFF profiling hook missing from this image."""
    import types
    try:
        import antenv
        from trn_agent_boot.trn_boot import _ntff_profile_via_ctypes
    except ImportError:
        return
    if "antenv.axon_hooks" in sys.modules:
        return
    mod = types.ModuleType("antenv.axon_hooks")
    mod._hook = _ntff_profile_via_ctypes("/opt/axon/libaxon_pjrt.so")
    mod.get_axon_ntff_profile_hook = lambda: mod._hook
    mod.set_axon_ntff_profile_hook = lambda h: setattr(mod, "_hook", h)
    sys.modules["antenv.axon_hooks"] = mod
    antenv.axon_hooks = mod


def kernel(**inputs):
    out, _ = run(trace=False, **inputs)
    return out


# revision 76
# speedup vs baseline: 1.0166x; 1.0166x over previous
"""GQA attention (BagleyAttention) on 8 Trainium2 NeuronCores.

Tensor-parallel over kv-head groups: core c owns kv head c and query heads
[4c, 4c+4). Each core computes its heads' attention and a partial output
projection [S, D]; the host sums the 8 partials.

v4: PSUM-pressure + ACT-overhead rework over v3 (403us -> ~389us):
  - projection rounds 1-3 run in two 256-column group-major passes with 6
    half-bank accumulators packed into 3 PSUM banks (a start=True matmul
    invalidates the FULL psum rows of its bank, so groups sharing a bank
    must accumulate contiguously -- never dcg-major); round 0 (x/weights
    still streaming in) runs full-width dc-major using the then-idle
    score/pv banks as 6 separate single-bank accumulators;
  - scores accumulate into a 2-bank [128,1024] PSUM tile; full k-chunks
    are exp'd in PAIRS with a single ACTIVATE (halves the 352-cycle fixed
    cost per instruction); in the projection-free final round the pab
    projection tile doubles as a second score pair-tile so the sc-write
    WAR never serializes against the exp chain;
  - the softmax denominator accumulates in ONE wide fp16 tile, DVE for
    pairs, idle GpSimd for the trailing diagonal chunks, so the Z-matmul
    never queues behind outproj copies; ob copies are split ACT/DVE per
    round with ACT kept clear wherever the exp chain is dense;
  - RoPE rotate-half runs on DVE stream_shuffle in a host-permuted head-dim
    basis (pairs (j, j+64) interleaved on adjacent partitions; swap mask
    i^1 is quadrant-local; the sign lives in the partner-indexed sin table
    and cos[j] == cos[j+64] for this rope) -> no PE permutation matmuls,
    no ACT PSUM->SBUF copies, and the projection accumulators are freed by
    two DVE reads;
  - outproj PSUM rotation folds idle projection banks in during the final
    rounds; x staging for round r+1 is queued right after round r's passB,
    ahead of the output-store DMAs.
"""

import math
import sys

sys.path.insert(0, "/opt/trn_rl_repo")

import numpy as np

# Problem sizes (hardcoded per contract; kernel.py reads no sibling files).
B, S, D = 1, 2048, 4096
H, KV, Dh = 32, 8, 128
G = H // KV            # query heads per kv head (= per core)
EH = G * Dh            # per-core q projection width (512)
N_CORES = 8

SB = 512               # s-block width (projection s-block = attention q-block)
HB = 256               # projection half-block (column pass width)
N_SB = S // SB         # 4 rounds
N_DC = D // 128        # 32 d-chunks
N_DCG = 8              # d-chunk groups of 4 (x staging granularity)
N_NB = D // SB         # 8 output d-blocks

EXP_BIAS = 9.5         # exp(s - EXP_BIAS); cancels in softmax normalization

SWAP_MASK = [i ^ 1 for i in range(32)]   # even/odd partition swap (per quad)

_cache = {}


def _build():
    import concourse.bass as bass
    import concourse.mybir as mybir
    import concourse.tile as tile
    from concourse import bacc
    from concourse.masks import make_identity

    dt = mybir.dt
    f32, f16 = dt.float32, dt.float16
    AF = mybir.ActivationFunctionType

    nc = bacc.Bacc("TRN2", target_bir_lowering=False, debug=False)

    # host-pretiled inputs (see _prep_inputs for layouts)
    xg = nc.dram_tensor("xg", [N_SB, N_DCG, 128, 4, SB], f16,
                        kind="ExternalInput").ap()
    wqt = nc.dram_tensor("wqt", [128, N_DC, EH], f16,
                         kind="ExternalInput").ap()
    wkt = nc.dram_tensor("wkt", [128, N_DC, Dh], f16,
                         kind="ExternalInput").ap()
    wvt = nc.dram_tensor("wvt", [128, N_DC, Dh], f16,
                         kind="ExternalInput").ap()
    wot = nc.dram_tensor("wot", [128, G, D], f16, kind="ExternalInput").ap()
    cosT = nc.dram_tensor("cosT", [Dh, S], f16, kind="ExternalInput").ap()
    sinT = nc.dram_tensor("sinT", [Dh, S], f16, kind="ExternalInput").ap()
    triT = nc.dram_tensor("triT", [128, 128], f16, kind="ExternalInput").ap()
    out = nc.dram_tensor("out", [S, D], f16, kind="ExternalOutput").ap()

    inv_sqrt_dh = 1.0 / math.sqrt(Dh)

    with tile.TileContext(nc) as tc, \
         tc.tile_pool(name="persist", bufs=1) as persist, \
         tc.tile_pool(name="projp", bufs=1, space="PSUM") as projp, \
         tc.tile_pool(name="scp", bufs=1, space="PSUM") as scp, \
         tc.tile_pool(name="pvp", bufs=1, space="PSUM") as pvp, \
         tc.tile_pool(name="trans", bufs=2, space="PSUM") as trans, \
         tc.tile_pool(name="xstage", bufs=10) as xstage, \
         tc.tile_pool(name="ropep", bufs=4) as ropep, \
         tc.tile_pool(name="expp", bufs=2) as expp, \
         tc.tile_pool(name="eaccp", bufs=2) as eaccp, \
         tc.tile_pool(name="miscp", bufs=2) as miscp, \
         tc.tile_pool(name="obuf", bufs=2) as obuf:

        # ---- long-lived SBUF tensors --------------------------------------
        qr = [[persist.tile([128, SB], f16, tag=f"qr{h}_{sb}",
                            name=f"qr{h}_{sb}") for sb in range(N_SB)]
              for h in range(G)]
        kr = [persist.tile([128, SB], f16, tag=f"kr{sb}", name=f"kr{sb}")
              for sb in range(N_SB)]
        vnat = [persist.tile([128, N_SB * Dh], f16, tag=f"vn{sb}",
                             name=f"vn{sb}") for sb in range(N_SB)]
        attn = [[persist.tile([128, SB], f16, tag=f"attn{h}_{t}",
                              name=f"attn{h}_{t}") for t in range(N_SB)]
                for h in range(G)]

        wq_h = persist.tile([128, N_DC, EH], f16, tag="wq_h", name="wq_h")
        wk_h = persist.tile([128, N_DC, Dh], f16, tag="wk_h", name="wk_h")
        wv_h = persist.tile([128, N_DC, Dh], f16, tag="wv_h", name="wv_h")
        wo_r = persist.tile([128, G, D], f16, tag="wo_r", name="wo_r")
        cos_sb = persist.tile([128, S], f16, tag="cos", name="cos_sb")
        sin_sb = persist.tile([128, S], f16, tag="sin", name="sin_sb")
        tri = persist.tile([128, 128], f16, tag="tri", name="tri")
        ones_h = persist.tile([128, 128], f16, tag="ones", name="ones_h")
        ident = persist.tile([128, 128], f16, tag="ident", name="ident")
        ebias = persist.tile([128, 1], f32, tag="ebias", name="ebias")

        nc.vector.memset(ones_h, 1.0)
        nc.vector.memset(ebias, -EXP_BIAS)
        make_identity(nc, ident)

        # PE warm-up: a few no-dep matmuls run during the initial DMA wait,
        # priming the HAM activity window so the first projection matmuls
        # start closer to the full 2.4 GHz clock. Measured sweep: more than
        # a handful costs real stream time (~285ns each) and is net-negative.
        wsrc = persist.tile([128, SB], f16, tag="wsrc", name="wsrc")
        nc.vector.memset(wsrc, 0.000244140625)
        warm = trans.tile([128, SB], f32, tag="tr", name="warm")
        for _ in range(6):
            nc.tensor.matmul(warm[:], ones_h[:], wsrc[:],
                             start=True, stop=True)

        vt_sb = persist.tile([128, SB], f16, tag="vt", name="vt_sb")

        # Projection accumulators: 6 groups x [128, HB] packed as 3 banks.
        # pab is a 2-bank tile (groups 0-3) so that in the projection-free
        # final rounds it can double as a SECOND score pair-tile.
        pab = projp.tile([128, 2 * SB], f32, tag="pab", name="pab")
        pc2 = projp.tile([128, SB], f32, tag="pc2", name="pc2")

        def acc_slot(g):
            if g < 4:
                return pab[:, g * HB:(g + 1) * HB]
            return pc2[:, (g - 4) * HB:(g - 3) * HB]

        # Scores: one 2-bank tile; chunk c uses half c%2.
        sc2 = scp.tile([128, 2 * SB], f32, tag="sc2", name="sc2")
        # pv accumulator (one bank, WAR-rotated across (t,h) blocks)
        pv = pvp.tile([128, SB], f32, tag="pv", name="pv")

        def sct(t, pi):
            # score pair-tile for chunk-pair index pi: alternate with the
            # (idle) pab banks in the final attention round so the sc WAR
            # never serializes against the exp chain
            if t == N_SB - 1 and pi % 2 == 1:
                return pab
            return sc2

        # -------------------------------------------------------------------
        # DMA emission (all host-pretiled, big per-partition rows)
        # -------------------------------------------------------------------
        xq_tiles = {}   # (sb, dcg) -> tile [128, 4, SB]

        def emit_x_dma(sb):
            for dcg in range(N_DCG):
                xq = xstage.tile([128, 4, SB], f16, tag="xq",
                                 name=f"xq{sb}_{dcg}")
                nc.sync.dma_start(out=xq, in_=xg[sb, dcg])
                xq_tiles[(sb, dcg)] = xq

        def emit_weight_dma():
            def wq_g(g):
                cs = slice(g * 8, (g + 1) * 8)
                nc.sync.dma_start(out=wq_h[:, cs, :], in_=wqt[:, cs, :])

            def wkv_g(g):
                cs = slice(g * 16, (g + 1) * 16)
                nc.sync.dma_start(out=wk_h[:, cs, :], in_=wkt[:, cs, :])
                nc.sync.dma_start(out=wv_h[:, cs, :], in_=wvt[:, cs, :])

            def xq_g(dcg):
                xq = xstage.tile([128, 4, SB], f16, tag="xq",
                                 name=f"xq0_{dcg}")
                nc.sync.dma_start(out=xq, in_=xg[0, dcg])
                xq_tiles[(0, dcg)] = xq

            # small first descriptors so the first matmul starts ASAP
            nc.sync.dma_start(out=wq_h[:, 0:1, :], in_=wqt[:, 0:1, :])
            nc.sync.dma_start(out=wk_h[:, 0:1, :], in_=wkt[:, 0:1, :])
            nc.sync.dma_start(out=wv_h[:, 0:1, :], in_=wvt[:, 0:1, :])
            xq_g(0)
            nc.sync.dma_start(out=wq_h[:, 1:2, :], in_=wqt[:, 1:2, :])
            nc.sync.dma_start(out=wk_h[:, 1:4, :], in_=wkt[:, 1:4, :])
            nc.sync.dma_start(out=wv_h[:, 1:4, :], in_=wvt[:, 1:4, :])
            nc.sync.dma_start(out=wq_h[:, 2:8, :], in_=wqt[:, 2:8, :])
            nc.sync.dma_start(out=wk_h[:, 4:8, :], in_=wkt[:, 4:8, :])
            nc.sync.dma_start(out=wv_h[:, 4:8, :], in_=wvt[:, 4:8, :])
            xq_g(1)
            nc.sync.dma_start(out=wk_h[:, 8:16, :], in_=wkt[:, 8:16, :])
            nc.sync.dma_start(out=wv_h[:, 8:16, :], in_=wvt[:, 8:16, :])
            wq_g(1); xq_g(2); xq_g(3)
            wq_g(2); wkv_g(1); xq_g(4); xq_g(5)
            wq_g(3); xq_g(6); xq_g(7)
            # trig/masks land before their first readers and must be EMITTED
            # before those readers too, else Tile orders the load after them.
            nc.sync.dma_start(out=cos_sb, in_=cosT)
            nc.sync.dma_start(out=sin_sb, in_=sinT)
            nc.sync.dma_start(out=tri, in_=triT)

        # -------------------------------------------------------------------
        # Projection pass: 6 groups (4 q heads, k, v) x 32 d-chunks over one
        # 256-column half. Group-major so group g's accumulator is complete
        # (and drainable) 5/6 of a pass before the pass ends.
        # -------------------------------------------------------------------
        def wsel(g, dc):
            if g < G:
                return wq_h[:, dc, g * 128:(g + 1) * 128]
            if g == 4:
                return wk_h[:, dc, :]
            return wv_h[:, dc, :]

        # NOTE on ordering: a start=True matmul invalidates the has_written
        # state for the FULL psum rows of its bank, so two accumulation
        # groups sharing a bank must never interleave their matmuls.
        # Group-major keeps each group contiguous (and lets its rope drain
        # run during the pass); it is the only legal order for the packed
        # half-bank accumulators.
        def emit_proj_pass(r, half):
            o = half * HB
            for g in range(6):
                dst = acc_slot(g)
                for dcg in range(N_DCG):
                    xf = xq_tiles[(r, dcg)]
                    for j in range(4):
                        dc = dcg * 4 + j
                        nc.tensor.matmul(dst, wsel(g, dc), xf[:, j, o:o + HB],
                                         start=(dc == 0),
                                         stop=(dc == N_DC - 1))

        # Round 0: x/weights stream in over ~20us, so group-major would
        # starve on DMA. With no attention live yet, the sc2/pv banks are
        # free: run the baseline-style full-width dc-major pass with 6
        # accumulators in 6 SEPARATE banks (cross-group interleave across
        # different banks is safe).
        def emit_proj_full_r0():
            accs = [pab[:, 0:SB], pab[:, SB:2 * SB], pc2[:],
                    sc2[:, 0:SB], sc2[:, SB:2 * SB], pv[:]]
            # dc-major through dc 27 (matches DMA arrival), then finish the
            # groups one at a time so their rope drains start staggered
            # instead of all at the pass boundary
            for dcg in range(N_DCG - 1):
                xf = xq_tiles[(0, dcg)]
                for j in range(4):
                    dc = dcg * 4 + j
                    for g in range(6):
                        nc.tensor.matmul(accs[g], wsel(g, dc), xf[:, j, :],
                                         start=(dc == 0), stop=False)
            xf = xq_tiles[(0, N_DCG - 1)]
            for g in range(6):
                for j in range(4):
                    dc = (N_DCG - 1) * 4 + j
                    nc.tensor.matmul(accs[g], wsel(g, dc), xf[:, j, :],
                                     start=False, stop=(dc == N_DC - 1))
            return accs

        # RoPE for one (slot, half): all DVE + one GpSimd add; the PSUM
        # accumulator is freed after the two DVE reads. sin_sb is the
        # PARTNER-indexed signed sin table so the multiply happens before
        # the (same-dtype) stream_shuffle:
        #   rot[p] = acc[p^1]*sinM[p] = shuffle(acc*sinP)[p],
        #   sinP[p] = sinM[p^1].
        def make_rope_units(r, half, accs=None, W=HB):
            o = half * HB
            ss = slice(r * SB + o, r * SB + o + W)

            def get_acc(i):
                return accs[i] if accs is not None else acc_slot(i)

            def rope(i):
                acc = get_acc(i)
                c_ = ropep.tile([128, W], f16, tag="rc", name=f"rc{r}_{i}")
                s_ = ropep.tile([128, W], f16, tag="rs", name=f"rs{r}_{i}")
                w_ = ropep.tile([128, W], f16, tag="rw", name=f"rw{r}_{i}")
                nc.vector.tensor_mul(c_, acc, cos_sb[:, ss])
                nc.vector.tensor_mul(s_, acc, sin_sb[:, ss])
                nc.vector.stream_shuffle(w_, s_, SWAP_MASK)
                dst = qr[i][r] if i < G else kr[r]
                nc.gpsimd.tensor_add(dst[:, o:o + W], c_, w_)

            def vdrain():
                nc.scalar.copy(out=vt_sb[:, o:o + W], in_=get_acc(5))

            return [lambda i=i: rope(i) for i in range(5)] + [vdrain]

        # V natural layout: transposes of one 128-col block each.
        # (XBAR DMA transposes were tried on both hwdge queues: the SP queue
        # blocks x/ob traffic behind the vdrain wait, and the ACT queue
        # mis-tracks the vt_sb WAR -> PE transposes it is.)
        def emit_transpose(r, blk):
            tp = trans.tile([128, 128], f16, tag="tr", name=f"tp{r}_{blk}")
            nc.tensor.transpose(tp[:], vt_sb[:, blk * 128:(blk + 1) * 128],
                                ident[:])
            nc.vector.tensor_copy(
                vnat[r][:, blk * Dh:(blk + 1) * Dh], tp[:])

        # -------------------------------------------------------------------
        # Attention for q-block t, head h. Chunks are emitted as units:
        #   ("pair", t, h, p): full chunks 2p, 2p+1 -> 2 sc MMs, 1 paired
        #       exp, 1 wide DVE e-add, 2 pv MMs
        #   ("solo", t, h, c): diagonal chunk -> sc MM, exp, mask, add, pv
        # -------------------------------------------------------------------
        att_state = {}

        def attn_begin(t, h):
            st = dict(n=4 * (t + 1))
            st["ed"] = eaccp.tile([128, 2 * SB], f16, tag="ed", name="ed")
            att_state[(t, h)] = st

        def attn_pair(t, h, p):
            st = att_state[(t, h)]
            sc = sct(t, p)
            c0 = 2 * p
            for ci in range(2):
                c = c0 + ci
                nc.tensor.matmul(
                    sc[:, ci * SB:(ci + 1) * SB],
                    kr[c // 4][:, (c % 4) * 128:(c % 4) * 128 + 128],
                    qr[h][t][:], start=True, stop=True)
            e = expp.tile([128, 2 * SB], f16, tag="e", name="e")
            nc.scalar.activation(e[:], sc[:], AF.Exp,
                                 scale=inv_sqrt_dh, bias=ebias[:])
            if p == 0:
                nc.vector.tensor_copy(st["ed"][:], e[:])
            else:
                nc.vector.tensor_add(st["ed"][:], st["ed"][:], e[:])
            for ci in range(2):
                c = c0 + ci
                vw = vnat[c // 4][:, (c % 4) * Dh:(c % 4 + 1) * Dh]
                nc.tensor.matmul(pv[:], vw, e[:, ci * SB:(ci + 1) * SB],
                                 start=(c == 0), stop=(c == st["n"] - 1))

        def attn_solo(t, h, c):
            st = att_state[(t, h)]
            sc = sct(t, c // 2)
            hf = (c % 2) * SB
            qlo = 128 * (c - 4 * t)
            W = SB - qlo
            nc.tensor.matmul(sc[:, hf:hf + W],
                             kr[c // 4][:, (c % 4) * 128:(c % 4) * 128 + 128],
                             qr[h][t][:, qlo:SB], start=True, stop=True)
            e = expp.tile([128, 2 * SB], f16, tag="e", name="e")
            nc.scalar.activation(e[:, hf:hf + W], sc[:, hf:hf + W], AF.Exp,
                                 scale=inv_sqrt_dh, bias=ebias[:])
            # diagonal: mask first 128 q-cols (on the pv critical path)
            nc.vector.tensor_mul(e[:, hf:hf + 128], e[:, hf:hf + 128], tri[:])
            ed = st["ed"]
            if c <= 1:
                # t == 0: ed halves first written by chunks 0 (full) / 1
                if c == 1:
                    nc.vector.memset(ed[:, SB:SB + 128], 0.0)
                nc.vector.tensor_copy(ed[:, hf + qlo:hf + SB], e[:, hf:hf + W])
            else:
                # in the proj-free final round GpSimd is idle: trailing solos
                # accumulate there so the Z-matmul doesn't queue behind the
                # DVE copies; in earlier rounds GpSimd runs the rope combines
                # and would be slower than DVE
                on_gp = (t == N_SB - 1) and c >= st["n"] - 2
                eng = nc.gpsimd if on_gp else nc.vector
                eng.tensor_add(ed[:, hf + qlo:hf + SB],
                               ed[:, hf + qlo:hf + SB], e[:, hf:hf + W])
            vw = vnat[c // 4][:, (c % 4) * Dh:(c % 4 + 1) * Dh]
            nc.tensor.matmul(pv[:, qlo:SB], vw, e[:, hf:hf + W],
                             start=(c == 0), stop=(c == st["n"] - 1))

        def attn_eplg(t, h):
            st = att_state.pop((t, h))
            zb = trans.tile([128, SB], f32, tag="tr", name=f"z{t}_{h}")
            nc.tensor.matmul(zb[:], ones_h[:], st["ed"][:, 0:SB],
                             start=True, stop=False)
            nc.tensor.matmul(zb[:], ones_h[:], st["ed"][:, SB:2 * SB],
                             start=False, stop=True)
            rz = miscp.tile([128, SB], f32, tag="rz", name="rz")
            nc.vector.reciprocal_approx_fast(out=rz, in_=zb[:])
            nc.vector.tensor_mul(attn[h][t][:], pv[:], rz[:])

        # -------------------------------------------------------------------
        # Output projection: 8 groups of 4 d-blocks per q-block; one DMA
        # per group ([128, 2048] = 4KB rows).
        # -------------------------------------------------------------------
        ob_state = {}

        def outproj_tile(t, i, ob_eng, deep_psum=0):
            sl = (i // N_NB) * 128          # s-offset within block
            nb = i % N_NB
            st_row = 4 * t + i // N_NB
            if t == 3 and i >= 24:
                # strict alternation at the very end: neither engine's queue
                # may delay the final copies ahead of the closing DMAs
                ob_eng = "act" if i % 2 else "dve"
            # in the proj-free rounds idle PSUM banks join the rotation so
            # the PE never waits on the ob copies. Round 4: pc2 (pab is the
            # alternate score tile there). Round 5: pc2 + both sc2 halves.
            if deep_psum == 2 and i % 5 == 2:
                op = pc2[:]
            elif deep_psum == 2 and i % 5 == 3:
                op = sc2[:, 0:SB]
            elif deep_psum == 2 and i % 5 == 4:
                op = sc2[:, SB:2 * SB]
            elif deep_psum == 1 and i % 3 == 2:
                op = pc2[:]
            else:
                op = trans.tile([128, SB], f32, tag="tr", name=f"op{t}_{i}")
            for hh in range(G):
                nc.tensor.matmul(op[:], attn[hh][t][:, sl:sl + 128],
                                 wo_r[:, hh, nb * SB:(nb + 1) * SB],
                                 start=(hh == 0), stop=(hh == G - 1))
            # group width: 4 d-blocks per DMA; narrower at the very end so
            # the final transfers start earlier (shorter kernel tail)
            w = 1 if (t == 3 and i >= 28) else 2 if (t == 3 and i >= 24) else 4
            if nb % w == 0:
                ob_state[st_row] = obuf.tile([128, w * SB], f16, tag="ob",
                                             name=f"ob{t}_{i}")
            ob = ob_state[st_row]
            qtr = nb % w
            if ob_eng == "act":
                nc.scalar.copy(out=ob[:, qtr * SB:(qtr + 1) * SB], in_=op[:])
            else:
                nc.vector.tensor_copy(ob[:, qtr * SB:(qtr + 1) * SB], op[:])
            if nb % w == w - 1:
                rs = slice(st_row * 128, (st_row + 1) * 128)
                cs = slice((nb - w + 1) * SB, (nb + 1) * SB)
                nc.sync.dma_start(out=out[rs, cs], in_=ob[:, 0:w * SB])
                del ob_state[st_row]

        # -------------------------------------------------------------------
        # Round schedule:
        #   r0: proj0|rope0       r1: proj1|rope1|attn0
        #   r2: proj2|rope2|attn1|outproj0   r3: proj3|rope3|attn2|outproj1
        #   r4: attn3|outproj2    r5: outproj3
        # -------------------------------------------------------------------
        # ob-copy share on ACT: ACT must keep slack for the exps (the sc2
        # pair rotation makes the PE's sc stream wait on exp completion);
        # round 4 is exp-densest so ACT gets no copies at all there
        ACT_OB_SHARE = {2: 0.4, 3: 0.45, 4: 0.0, 5: 0.55}

        def emit_round(r):
            ta = r - 1            # attention q-block this round
            to = r - 2            # out-projection q-block this round

            # attention PE-work units for this round, in order
            units = []
            if 0 <= ta < N_SB:
                n = 4 * (ta + 1)
                for h in range(G):
                    units.append(("begin", ta, h))
                    for p in range(2 * ta):
                        units.append(("pair", ta, h, p))
                    for c in range(4 * ta, n):
                        units.append(("solo", ta, h, c))
                    units.append(("eplg", ta, h))
            nop = 32 if 0 <= to < N_SB else 0
            nch = sum(1 for u in units if u[0] in ("pair", "solo"))

            ropes0 = []
            ropesA = []
            if r == 0:
                accs0 = emit_proj_full_r0()
                emit_x_dma(1)
                ropes0 = make_rope_units(0, 0, accs=accs0, W=SB)
            elif r < N_SB:
                emit_proj_pass(r, 0)
                ropesA = make_rope_units(r, 0)

            # Post-passA stream. With group-major passes, rope unit g's PSUM
            # deps complete (g+1)/6 of the way through the pass, so rope
            # units emitted before passB execute DURING the pass itself and
            # passB never waits on accumulator drains.
            seq = []
            ui = 0
            if r == 0:
                # v transposes deferred into round 1: they wait on the ACT
                # vdrain and would stall the PE right before round 1's passA
                seq += [("rope0", k) for k in range(6)]
            elif r < N_SB:
                # round 0's deferred transposes must precede ropeA: round
                # 1's vdrain overwrites the shared vt_sb staging buffer
                if r == 1:
                    seq += [("transp", 0, b) for b in range(4)]
                seq += [("ropeA", k) for k in range(6)]
                seq.append(("passB", r))
                seq.append(("transp", r, 0))
                seq.append(("ropesB_make", r))
                # interleave ropeB with the first attn units so the v-half1
                # transposes (which need ropeB's vdrain) come a bit later
                for k in range(6):
                    seq.append(("ropeB", k))
                    if ui < len(units):
                        seq.append(units[ui]); ui += 1
                seq.append(("transp", r, 1))
                seq.append(("transp", r, 2))
                seq.append(("transp", r, 3))
            op_i = 0
            ob_flip = 0.0
            chunk_seen = sum(1 for u in seq if u[0] in ("pair", "solo"))
            while ui < len(units):
                u = units[ui]; ui += 1
                if u[0] == "eplg":
                    # cover the Z-matmul's E-accumulator wait; force these
                    # copies onto ACT so the DVE queue stays short ahead of
                    # the Z-matmul's ed dependency
                    for _ in range(2):
                        if op_i < nop:
                            seq.append(("opact", to, op_i))
                            op_i += 1
                seq.append(u)
                if u[0] in ("pair", "solo"):
                    chunk_seen += 1
                    while nch and op_i < nop and (op_i + 1) / nop <= \
                            chunk_seen / nch:
                        seq.append(("op", to, op_i))
                        op_i += 1
            while op_i < nop:
                seq.append(("op", to, op_i))
                op_i += 1

            act_share = ACT_OB_SHARE.get(r, 0.0)
            ropesB = []
            for u in seq:
                kind = u[0]
                if kind == "begin":
                    attn_begin(u[1], u[2])
                elif kind == "pair":
                    attn_pair(u[1], u[2], u[3])
                elif kind == "solo":
                    attn_solo(u[1], u[2], u[3])
                elif kind == "eplg":
                    attn_eplg(u[1], u[2])
                elif kind == "passB":
                    emit_proj_pass(u[1], 1)
                    # queue next round's x staging ahead of this round's
                    # output-store DMAs so the next passA never starves
                    if u[1] + 1 < N_SB:
                        emit_x_dma(u[1] + 1)
                    if u[1] == 1:
                        for g in range(G):
                            nc.sync.dma_start(out=wo_r[:, g, :],
                                              in_=wot[:, g, :])
                elif kind == "rope0":
                    ropes0[u[1]]()
                elif kind == "ropeA":
                    ropesA[u[1]]()
                elif kind == "ropesB_make":
                    ropesB[:] = make_rope_units(u[1], 1)
                elif kind == "ropeB":
                    ropesB[u[1]]()
                elif kind == "transp":
                    emit_transpose(u[1], u[2])
                elif kind == "opact":
                    outproj_tile(u[1], u[2], "dve",
                                 deep_psum=max(0, r - N_SB + 1))
                elif kind == "op":
                    ob_flip += act_share
                    if ob_flip >= 1.0:
                        ob_flip -= 1.0
                        eng = "act"
                    else:
                        eng = "dve"
                    outproj_tile(u[1], u[2], eng,
                                 deep_psum=max(0, r - N_SB + 1))

        emit_weight_dma()
        for r in range(N_SB + 2):
            emit_round(r)
            # pop x staging after the round that consumed it
            if r < N_SB:
                for dcg in range(N_DCG):
                    xq_tiles.pop((r, dcg), None)

    nc.compile()
    return nc


def _prep_inputs(hidden_states, Wq, Wk, Wv, Wo, cos, sin):
    x = np.asarray(hidden_states, dtype=np.float32).reshape(S, D)
    Wq = np.asarray(Wq, dtype=np.float32)
    Wk = np.asarray(Wk, dtype=np.float32)
    Wv = np.asarray(Wv, dtype=np.float32)
    Wo = np.asarray(Wo, dtype=np.float32)
    cos = np.asarray(cos, dtype=np.float32)
    sin = np.asarray(sin, dtype=np.float32)

    # Head-dim basis permutation: partition 2j holds element j, partition
    # 2j+1 holds element j+64 -> the rope pair sits on adjacent partitions
    # and rotate-half becomes a quadrant-local even/odd stream_shuffle.
    half = Dh // 2
    perm = np.empty(Dh, dtype=np.int64)
    perm[0::2] = np.arange(half)
    perm[1::2] = np.arange(half) + half

    # x pretiled: xg[sb, dcg, p, j, s] = x.T[dcg*512 + j*128 + p, sb*512 + s]
    xT = np.ascontiguousarray(x.T).astype(np.float16)
    xg = np.ascontiguousarray(
        xT.reshape(N_DCG, 4, 128, N_SB, SB).transpose(3, 0, 2, 1, 4))
    # cos in permuted basis; sin with the rotate-half sign folded in:
    # row 2j   (elem j):    needs -sin[j]   * (partner value)
    # row 2j+1 (elem j+64): needs +sin[j+64]* (partner value)
    # The kernel multiplies BEFORE shuffling, so ship the partner-indexed
    # table sinP[p] = sinM[p^1].
    cosM = cos.T[perm, :]
    sinM = sin.T[perm, :].copy()
    sinM[0::2, :] *= -1.0
    swap = np.arange(Dh) ^ 1
    sinP = sinM[swap, :]
    cosT_h = np.ascontiguousarray(cosM).astype(np.float16)
    sinT_h = np.ascontiguousarray(sinP).astype(np.float16)
    # lower-triangle (inclusive) 0/1 mask for the 128x128 diagonal block
    kp = np.arange(128)[:, None]
    qc = np.arange(128)[None, :]
    triT = (kp <= qc).astype(np.float16)

    in_maps = []
    for c in range(N_CORES):
        wq_s = Wq[c * EH:(c + 1) * EH, :]          # [EH, D]
        wk_s = Wk[c * Dh:(c + 1) * Dh, :]
        wv_s = Wv[c * Dh:(c + 1) * Dh, :]
        wo_s = Wo[:, c * EH:(c + 1) * EH]          # [D, EH]
        # permute q/k head-dim rows into the interleaved rope basis
        wq_p = wq_s.reshape(G, Dh, D)[:, perm, :].reshape(EH, D)
        wk_p = wk_s[perm, :]
        # wqt[p, dc, e] = wq_p.T[dc*128+p, e]
        wqt = np.ascontiguousarray(
            np.ascontiguousarray(wq_p.T).astype(np.float16)
            .reshape(N_DC, 128, EH).transpose(1, 0, 2))
        wkt = np.ascontiguousarray(
            np.ascontiguousarray(wk_p.T).astype(np.float16)
            .reshape(N_DC, 128, Dh).transpose(1, 0, 2))
        wvt = np.ascontiguousarray(
            np.ascontiguousarray(wv_s.T).astype(np.float16)
            .reshape(N_DC, 128, Dh).transpose(1, 0, 2))
        # wot[p, h, d] = wo_s.T[h*128+p, d]
        wot = np.ascontiguousarray(
            np.ascontiguousarray(wo_s.T).astype(np.float16)
            .reshape(G, 128, D).transpose(1, 0, 2))
        in_maps.append({
            "xg": xg, "wqt": wqt, "wkt": wkt, "wvt": wvt, "wot": wot,
            "cosT": cosT_h, "sinT": sinT_h, "triT": triT,
        })
    return in_maps


def run(trace=False, **inputs):
    """Run on hardware; returns (full_output, exec_time_ns or None)."""
    from concourse.bass_utils import run_bass_kernel_spmd

    if trace:
        _install_ntff_hook()
    if "nc" not in _cache:
        _cache["nc"] = _build()
    nc = _cache["nc"]
    in_maps = _prep_inputs(**inputs)
    res = run_bass_kernel_spmd(nc, in_maps, core_ids=list(range(N_CORES)),
                               trace=trace)
    acc = res.results[0]["out"].astype(np.float32)
    for c in range(1, N_CORES):
        acc += res.results[c]["out"]
    return acc.reshape(B, S, D), res.exec_time_ns


def _install_ntff_hook():
    """Register the axon NTFF profiling hook missing from this image."""
    import types
    try:
        import antenv
        from trn_agent_boot.trn_boot import _ntff_profile_via_ctypes
    except ImportError:
        return
    if "antenv.axon_hooks" in sys.modules:
        return
    mod = types.ModuleType("antenv.axon_hooks")
    mod._hook = _ntff_profile_via_ctypes("/opt/axon/libaxon_pjrt.so")
    mod.get_axon_ntff_profile_hook = lambda: mod._hook
    mod.set_axon_ntff_profile_hook = lambda h: setattr(mod, "_hook", h)
    sys.modules["antenv.axon_hooks"] = mod
    antenv.axon_hooks = mod


def kernel(**inputs):
    out, _ = run(trace=False, **inputs)
    return out
